# revision 4
# baseline (speedup 1.0000x reference)
"""Trainium2 Bass kernel for nn_ClusteringLoss (k-means, K=32, D=64, 10 iters).

Self-contained: host-side sharding/layout prep (numpy + jax-cpu for the init
permutation) + a Bass/Tile device kernel run via run_bass_kernel_spmd on 8
NeuronCores (data-parallel over the 500k points).

Per-core device pipeline (F points, padded; all resident in SBUF):
  B-stack [128, F] bf16   rows 0-63 = bf16-hi(x).T, rows 64-127 = bf16-mid.T
  A-tiles [128, FT*65] bf16  per-128-point tile [xh | valid], rows permuted to
                             match the transpose-reduce output layout
  Per iteration:
    PE   scores s = -2 x.c (2 contract-128 bf16 matmuls per 512-pt chunk,
         4-way col-tiled into PSUM [128,512] banks; partition 32b+k = cluster)
    ACT  += (cc_k + 16384) per-partition bias in-place -> quantize to 2^-9 grid
    ACT  += -16128 in-place -> v = 256 + s_grid
    PE   += (k-16)*2^-15 const-matmul -> index packed below the grid
    DVE  fused 32x32-transpose min-reduce -> per-point packed min v* (A-layout)
    DVE  extract cl = v*-bits; one-hot O = (iota == cl) bf16
    PE   segment matmul [csum | counts] += O.T @ [xh | valid] (col-tiled)
    AllReduce [K,65] over 8 cores; c' = csum/max(N,1); rebuild c-constants.
  Final pass: plain cc bias (no grid) -> exact per-point min -> host assembles
  loss = mean(xx) + mean(min); cl un-permuted on host.
"""
import numpy as np
import ml_dtypes

import concourse.bass as bass
import concourse.tile as tile
import concourse.bacc as bacc
from concourse import mybir
from concourse.bass_utils import run_bass_kernel_spmd

BF = mybir.dt.bfloat16
F32 = mybir.dt.float32
AF = mybir.ActivationFunctionType
OP = mybir.AluOpType

K = 32
D = 64
NITER = 10
NCORES = 8

GRID_BIG = 24576.0     # quantize-add: mid-binade [16384,32768), ULP 2^-9 grid
RESCALE = -24320.0     # -(24576 - 256): v -> 256 + s_grid, ULP <= 2^-15
EPS = 2.0 ** -15       # index packing step
KOFF = 16.0

IDMASK = list(range(32))


def _perm_map(F):
    """tile tau=(bank*16+t), row p=32b+q  ->  shard point index."""
    nbank = F // 2048
    p = np.arange(128)
    pt = (np.arange(nbank)[:, None, None] * 2048)
    pt = (np.arange(nbank)[:, None, None] * 2048
          + 512 * (p[None, None, :] // 32)
          + 32 * np.arange(16)[None, :, None]
          + (p[None, None, :] % 32))          # [nbank, 16, 128]
    return pt.reshape(F // 128, 128)


def _host_prep(Z_full, F):
    N = Z_full.shape[0]
    shard_n = N // NCORES
    ntile = F // 128
    pt = _perm_map(F)
    bstacks, atiles = [], []
    for c in range(NCORES):
        shard = np.zeros((F, D), np.float32)
        shard[:shard_n] = Z_full[c * shard_n:(c + 1) * shard_n]
        valid = np.zeros(F, np.float32)
        valid[:shard_n] = 1.0
        xh = shard.astype(ml_dtypes.bfloat16)
        xm = (shard - xh.astype(np.float32)).astype(ml_dtypes.bfloat16)
        bs = np.empty((128, F), dtype=ml_dtypes.bfloat16)
        bs[:64] = xh.T
        bs[64:] = xm.T
        bstacks.append(bs)
        at = np.empty((128, ntile, 65), dtype=ml_dtypes.bfloat16)
        at[:, :, :64] = xh[pt].transpose(1, 0, 2)
        at[:, :, 64] = valid[pt].T
        atiles.append(np.ascontiguousarray(at.reshape(128, ntile * 65)))
    return bstacks, atiles, pt


def _build_program(F):
    nbank = F // 2048
    ntile = F // 128

    nc = bacc.Bacc("TRN2", target_bir_lowering=False, debug=False,
                   num_devices=NCORES)

    a_bstack = nc.dram_tensor("bstack", [128, F], BF, kind="ExternalInput").ap()
    a_atile = nc.dram_tensor("atile", [128, ntile * 65], BF,
                             kind="ExternalInput").ap()
    a_c0 = nc.dram_tensor("c0", [K, D], F32, kind="ExternalInput").ap()
    a_iota = nc.dram_tensor("iota", [128, K], BF, kind="ExternalInput").ap()
    a_kepat = nc.dram_tensor("kepat", [1, 128], BF, kind="ExternalInput").ap()
    a_ones = nc.dram_tensor("onesrow", [1, 512], BF, kind="ExternalInput").ap()

    o_cl = nc.dram_tensor("cl_out", [128, ntile], F32, kind="ExternalOutput").ap()
    o_mfin = nc.dram_tensor("mfin_out", [128, ntile], F32,
                            kind="ExternalOutput").ap()
    o_stat = nc.dram_tensor("stat_out", [K, 66], F32, kind="ExternalOutput").ap()

    cc_ins = [nc.dram_tensor(f"cc_in{i}", [K, 66], F32) for i in range(NITER)]
    cc_outs = [nc.dram_tensor(f"cc_out{i}", [K, 66], F32, addr_space="Shared")
               for i in range(NITER)]

    with tile.TileContext(nc) as tc:
        with tc.tile_pool(name="big", bufs=1) as big, \
             tc.tile_pool(name="small", bufs=1) as small, \
             tc.tile_pool(name="work", bufs=2) as work, \
             tc.tile_pool(name="oh", bufs=3) as ohpool, \
             tc.tile_pool(name="ps", bufs=6, space="PSUM") as ps, \
             tc.tile_pool(name="psseg", bufs=2, space="PSUM") as psseg:

            t_bstack = big.tile([128, F], BF)
            t_atile = big.tile([128, ntile * 65], BF)
            t_iota = small.tile([128, K], BF)
            t_kepat = small.tile([1, 128], BF)
            t_ones = small.tile([1, 512], BF)
            t_v = big.tile([128, ntile], F32)
            t_cl = big.tile([128, ntile], BF)
            t_clout = big.tile([128, ntile], F32)
            t_tmp = big.tile([128, ntile], F32)
            t_statout = small.tile([K, 66], F32)

            # chunked B-stack load for early compute overlap
            CH = max(1, nbank // 8) * 2048
            for off in range(0, F, CH):
                w = min(CH, F - off)
                nc.sync.dma_start(t_bstack[:, off:off + w],
                                  a_bstack[:, off:off + w])
            nc.sync.dma_start(t_atile[:], a_atile)
            nc.sync.dma_start(t_iota[:], a_iota)
            nc.sync.dma_start(t_kepat[:], a_kepat)
            nc.sync.dma_start(t_ones[:], a_ones)

            t_c = small.tile([K, D], F32)
            nc.sync.dma_start(t_c[:], a_c0)
            t_resc = small.tile([128, 1], F32)
            nc.gpsimd.memset(t_resc[:], RESCALE)

            def centroid_derived(tag):
                # cT [64, 32]: block-transpose then move block1 down
                t_ct0 = work.tile([32, 64], F32, name=f"ct0_{tag}", tag="ct0")
                nc.vector.transpose(t_ct0[:], t_c[:])
                t_cT = work.tile([64, K], F32, name=f"cT_{tag}", tag="cT")
                nc.vector.tensor_copy(t_cT[0:32, :], t_ct0[:, 0:32])
                nc.vector.stream_shuffle(t_cT[32:64, :], t_ct0[:, 32:64], IDMASK)
                t_m2cT = work.tile([64, K], F32, name=f"m2_{tag}", tag="m2")
                nc.vector.tensor_scalar_mul(t_m2cT[:], t_cT[:], -1.0)
                t_ch = work.tile([64, K], BF, name=f"ch_{tag}", tag="ch")
                nc.vector.tensor_copy(t_ch[:], t_m2cT[:])
                t_chf = work.tile([64, K], F32, name=f"chf_{tag}", tag="chf")
                nc.vector.tensor_copy(t_chf[:], t_ch[:])
                t_cm = work.tile([64, K], BF, name=f"cm_{tag}", tag="cm")
                nc.vector.tensor_tensor(t_cm[:], t_m2cT[:], t_chf[:],
                                        op=OP.subtract)
                t_stat1 = work.tile([128, K], BF, name=f"s1_{tag}", tag="s1")
                t_stat2 = work.tile([128, K], BF, name=f"s2_{tag}", tag="s2")
                nc.vector.tensor_copy(t_stat1[0:64, :], t_ch[:])
                nc.vector.tensor_copy(t_stat2[0:64, :], t_cm[:])
                for q in range(2):
                    nc.vector.stream_shuffle(t_stat1[64 + 32 * q:96 + 32 * q, :],
                                             t_ch[32 * q:32 * q + 32, :], IDMASK)
                    nc.vector.stream_shuffle(t_stat2[64 + 32 * q:96 + 32 * q, :],
                                             t_cm[32 * q:32 * q + 32, :], IDMASK)
                t_csq = work.tile([K, D], F32, name=f"csq_{tag}", tag="csq")
                nc.vector.tensor_tensor(t_csq[:], t_c[:], t_c[:], op=OP.mult)
                t_cc = work.tile([K, 1], F32, name=f"cc_{tag}", tag="cc")
                nc.vector.reduce_sum(t_cc[:], t_csq[:], axis=mybir.AxisListType.X)
                t_ccq = work.tile([128, 1], F32, name=f"ccq_{tag}", tag="ccq")
                nc.vector.tensor_copy(t_ccq[0:32, :], t_cc[:])
                for q in range(1, 4):
                    nc.vector.stream_shuffle(t_ccq[32 * q:32 * q + 32, :],
                                             t_cc[:, :], IDMASK)
                # bias = cc/2 + GRID_BIG ; final-pass bias = cc/2
                t_bias = work.tile([128, 1], F32, name=f"bias_{tag}", tag="bias")
                nc.vector.tensor_scalar(t_bias[:], t_ccq[:], 0.5, GRID_BIG,
                                        op0=OP.mult, op1=OP.add)
                t_cch = work.tile([128, 1], F32, name=f"cch_{tag}", tag="cch")
                nc.vector.tensor_scalar_mul(t_cch[:], t_ccq[:], 0.5)
                return t_stat1, t_stat2, t_bias, t_cch

            def iteration(it, t_stat1, t_stat2, t_bias, final=False):
                if not final:
                    t_seg = psseg.tile([128, 65], F32, name=f"seg{it}", tag="seg")
                for bk in range(nbank):
                    t_b = ps.tile([128, 512], F32, name=f"b{it}_{bk}", tag="db")
                    base = bk * 2048
                    for j in range(4):
                        rhs = t_bstack[:, base + 512 * j: base + 512 * (j + 1)]
                        pb = t_b[32 * j:32 * j + 32, :]
                        nc.tensor.matmul(pb, lhsT=t_stat1[:], rhs=rhs,
                                         start=True, stop=False,
                                         tile_position=(0, 32 * j))
                        nc.tensor.matmul(pb, lhsT=t_stat2[:], rhs=rhs,
                                         start=False, stop=final,
                                         tile_position=(0, 32 * j))
                    nc.scalar.activation(t_b[:], t_b[:], AF.Identity,
                                         bias=t_bias[:, 0:1])
                    if not final:
                        nc.scalar.activation(t_b[:], t_b[:], AF.Identity,
                                             bias=t_resc[:, 0:1])
                        nc.tensor.matmul(t_b[:], lhsT=t_kepat[0:1, :],
                                         rhs=t_ones[0:1, :],
                                         start=False, stop=True,
                                         skip_group_check=True)
                    nc.vector.tensor_reduce(
                        t_v[:, bk * 16:(bk + 1) * 16],
                        t_b[:].rearrange("p (t k) -> p t k", k=32),
                        axis=mybir.AxisListType.X, op=OP.min,
                        apply_transpose=True)
                if final:
                    return
                EX = 128
                for g in range(0, ntile, EX):
                    sl = slice(g, min(g + EX, ntile))
                    vv = t_v[:, sl]
                    tt = t_tmp[:, sl]
                    nc.vector.tensor_scalar(tt[:], vv[:], 512.0, 2.0 ** 23,
                                            op0=OP.mult, op1=OP.add)
                    nc.vector.tensor_scalar_add(tt[:], tt[:], -(2.0 ** 23))
                    nc.vector.tensor_scalar(tt[:], tt[:], -64.0, KOFF,
                                            op0=OP.mult, op1=OP.add)
                    nc.vector.scalar_tensor_tensor(tt[:], vv[:], 2.0 ** 15,
                                                   tt[:], op0=OP.mult,
                                                   op1=OP.add)
                    nc.vector.tensor_copy(t_cl[:, sl], tt[:])
                if it == NITER - 1:
                    nc.vector.tensor_copy(t_clout[:], t_tmp[:])
                NT16 = 16
                for ch in range(ntile // NT16):
                    t_oh = ohpool.tile([128, NT16, K], BF,
                                       name=f"oh{it}_{ch}", tag="ohc")
                    cl_b = t_cl[:, ch * NT16:(ch + 1) * NT16].rearrange(
                        "p (t k) -> p t k", k=1).broadcast_to([128, NT16, K])
                    io_b = t_iota[:].rearrange(
                        "p (t k) -> p t k", t=1).broadcast_to([128, NT16, K])
                    nc.vector.tensor_tensor(t_oh[:], io_b, cl_b, op=OP.is_equal)
                    for t4 in range(0, NT16, 4):
                        for j in range(4):
                            tau = ch * NT16 + t4 + j
                            nc.tensor.matmul(
                                t_seg[32 * j:32 * j + 32, :],
                                lhsT=t_oh[:, t4 + j, :],
                                rhs=t_atile[:, tau * 65:(tau + 1) * 65],
                                start=(tau < 4), stop=(tau >= ntile - 4),
                                tile_position=(0, 32 * j),
                                skip_group_check=True)
                t_segs = work.tile([128, 65], F32, name=f"sgs{it}", tag="sgs")
                nc.vector.tensor_copy(t_segs[:], t_seg[:])
                t_sega = work.tile([K, 66], F32, name=f"sga{it}", tag="sga")
                nc.gpsimd.memset(t_sega[:], 0.0)
                nc.vector.tensor_copy(t_sega[:, 0:65], t_segs[0:32, :])
                t_shuf = work.tile([K, 65], F32, name=f"shf{it}", tag="shf")
                for j in range(1, 4):
                    nc.vector.stream_shuffle(t_shuf[:],
                                             t_segs[32 * j:32 * j + 32, :],
                                             IDMASK)
                    nc.vector.tensor_tensor(t_sega[:, 0:65], t_sega[:, 0:65],
                                            t_shuf[:], op=OP.add)
                nc.sync.dma_start(cc_ins[it][:], t_sega[:])
                nc.gpsimd.collective_compute(
                    "AllReduce", OP.add,
                    replica_groups=[list(range(NCORES))],
                    ins=[cc_ins[it][:]], outs=[cc_outs[it][:]])
                t_red = work.tile([K, 66], F32, name=f"red{it}", tag="red")
                nc.sync.dma_start(t_red[:], cc_outs[it][:])
                if it == NITER - 1:
                    nc.vector.tensor_copy(t_statout[:], t_red[:])
                t_cnt = work.tile([K, 1], F32, name=f"cnt{it}", tag="cnt")
                nc.vector.tensor_scalar_max(t_cnt[:], t_red[:, 64:65], 1.0)
                t_rcp = work.tile([K, 1], F32, name=f"rcp{it}", tag="rcp")
                nc.vector.reciprocal(t_rcp[:], t_cnt[:])
                nc.vector.tensor_scalar(t_c[:], t_red[:, 0:64], t_rcp[:, 0:1],
                                        None, op0=OP.mult)

            for it in range(NITER):
                s1, s2, bias, ccq = centroid_derived(f"i{it}")
                iteration(it, s1, s2, bias)
            s1, s2, bias, ccq = centroid_derived("fin")
            iteration(NITER, s1, s2, ccq, final=True)

            nc.sync.dma_start(o_cl, t_clout[:])
            nc.sync.dma_start(o_mfin, t_v[:])
            nc.sync.dma_start(o_stat, t_statout[:])

    nc.compile()
    return nc


_CACHE = {}


def _run(Z, F, trace=False):
    import jax
    N = Z.shape[0]
    cpu = jax.devices("cpu")[0]
    with jax.default_device(cpu):
        idx = np.asarray(jax.random.permutation(jax.random.key(1), N)[:K])
    c0 = np.ascontiguousarray(Z[idx].astype(np.float32))

    bstacks, atiles, pt = _host_prep(Z.astype(np.float32), F)
    iota_np = np.tile(np.arange(K, dtype=ml_dtypes.bfloat16)[None, :], (128, 1))
    kepat_np = (((np.arange(128) % K) - KOFF) * EPS).astype(
        ml_dtypes.bfloat16)[None, :]
    ones_np = np.ones((1, 512), dtype=ml_dtypes.bfloat16)

    if F not in _CACHE:
        _CACHE[F] = _build_program(F)
    nc = _CACHE[F]

    in_maps = [{
        "bstack": np.asarray(bstacks[c]),
        "atile": np.asarray(atiles[c]),
        "c0": c0,
        "iota": iota_np,
        "kepat": kepat_np,
        "onesrow": ones_np,
    } for c in range(NCORES)]
    res = run_bass_kernel_spmd(nc, in_maps, list(range(NCORES)), trace=trace)

    shard_n = N // NCORES
    cl_full = np.empty(N, np.int32)
    min_sum = 0.0
    flat = pt.reshape(-1)
    for c in range(NCORES):
        r = res.results[c]
        cl_shard = np.empty(F, np.float32)
        mf_shard = np.empty(F, np.float32)
        cl_shard[flat] = r["cl_out"].T.reshape(-1)
        mf_shard[flat] = r["mfin_out"].T.reshape(-1)
        cl_full[c * shard_n:(c + 1) * shard_n] = \
            cl_shard[:shard_n].astype(np.int32)
        min_sum += mf_shard[:shard_n].sum(dtype=np.float64)

    Z64 = Z.astype(np.float64)
    loss = np.float32(((Z64 * Z64).sum() + 2.0 * min_sum) / N)
    return (loss, cl_full), res


def kernel(Z):
    (loss, cl), _ = _run(np.asarray(Z), 63488)
    return loss, cl
